# revision 50
# baseline (speedup 1.0000x reference)
"""Trainium2 Bass kernel for DiagonalGMMPosterior (vq_codebook).

Reference computation (per batch b, descriptor n, cluster k):
    dist[k,n]  = sum_d (x[d,n] - mu_n[k,d])^2 * exp(-log_sigma[k,d])
    logits     = -dist + log_alpha[k] - 0.5 * sum_d log_sigma[k,d]
    out[k,n]   = softmax_k(logits)

Device strategy (8 NeuronCores, data-parallel over the batch axis):
  * Host folds all (K,D) parameter math into GEMM coefficients, CENTERS
    them across K (softmax is shift invariant per column -> logits stay
    within ~+-16, no per-n max pass), then shifts by -7 so exp() fits
    comfortably in fp16 (max e ~ e^9; values that underflow fp16 have
    posterior < 1e-4, far under tolerance).
  * The GEMM uses two quadratic bases computed from x in one elementwise
    pass each, avoiding any separate f32->f16 conversion of x:
        q = x^2           (ScalarE Square, fp16 out)
        u = x^2 + 0.5 x   (DVE/GpSimd scalar_tensor_tensor, fp16 out)
    logits = sum_d (a1-2*a2)*q + (2*a2)*u + cc.
  * All matmuls are fp16 (16-bit stationaries share LDWEIGHTS across
    matmuls; f32r must reload per-matmul which serializes TensorE).
  * K=64 uses half the 128 lanes and ScalarE/VectorE cost is per-COLUMN,
    so each iteration processes TWO 1024-column tiles stacked on the
    partition axis: tile A's dist GEMM uses zero-padded stationaries
    [w;0] (PE cols 0-63), tile B [0;w] (PE cols 64-127), accumulating
    into the SAME PSUM banks.  The whole epilogue (exp, block-diag ones
    denominator matmul, reciprocal, normalize) runs on [128, 1024] tiles.
  * The reciprocal writes fp16 directly (RECIPROCAL_APPROX_FAST via
    _custom_dve; the fp32-only wrapper assert guards the in-pipe bit
    trick, the output cast is the standard DVE write port) so the final
    multiply is all-16-bit and hits the DVE 2x mode.
"""

import ml_dtypes
import numpy as np

import concourse.bacc as bacc
import concourse.bass as bass
import concourse.tile as tile
from concourse import mybir
from concourse.bass_utils import run_bass_kernel_spmd
from concourse.dve_ops import RECIP_APPROX_FAST_CONSTS, RECIPROCAL_APPROX_FAST

B, D, N, K = 16, 128, 16384, 64
NCORES = 8
BPC = B // NCORES   # batches per core
NT = 512            # one PSUM bank of fp32
PAIR = 2 * NT       # psum tile width (columns shared by the stacked halves)
STEP = 2 * PAIR     # x columns consumed per iteration (two 1024-col tiles)

F32 = mybir.dt.float32
F32R = mybir.dt.float32r
F16 = mybir.dt.float16

_CACHE = {}


def _build_nc():
    # Bacc (not raw Bass): its compile() pass legalizes Tile's multi-wait
    # instructions (move_matmul_waits_to_ldweights + generate_event_semaphores)
    # down to the 1-wait-per-instruction hardware limit.
    nc = bacc.Bacc("TRN2", target_bir_lowering=False, debug=False)
    x_in = nc.declare_dram_parameter("x", [BPC, D, N], F32R, isOutput=False)
    # four zero-padded fp16 stationaries: [wq;0], [wu;0], [0;wq], [0;wu]
    w_in = nc.declare_dram_parameter("w", [D, 4, 2 * K], F16, isOutput=False)
    cc_in = nc.declare_dram_parameter("cc", [2 * K, 1], F32, isOutput=False)
    ones_in = nc.declare_dram_parameter(
        "ones_bd", [2 * K, 2 * K], F16, isOutput=False
    )
    # fp16 output halves the store traffic at ~5e-4 rounding (posteriors
    # live in [0,1]); the host widens back to fp32
    out_ext = nc.declare_dram_parameter("out", [BPC, K, N], F16, isOutput=True)

    with tile.TileContext(nc) as tc:
        with (
            tc.tile_pool(name="consts", bufs=1) as consts,
            tc.tile_pool(name="xtp", bufs=12) as xtp,
            tc.tile_pool(name="xp", bufs=6) as xp,
            tc.tile_pool(name="ep", bufs=6) as ep,
            tc.tile_pool(name="op", bufs=6) as op,
            tc.tile_pool(name="rp", bufs=6) as rp,
            tc.tile_pool(name="pd", bufs=4, space="PSUM") as pdp,
            tc.tile_pool(name="pb", bufs=4, space="PSUM") as pbp,
        ):
            # x loads for the first two iterations go on the SP queue BEFORE
            # the consts so the first compute starts ~2us earlier; the
            # consts are only needed by the matmul/exp stages
            xt_pre = []
            for i in range(2):
                xt = xtp.tile([D, STEP], F32R, tag="xt")
                nc.sync.dma_start(out=xt, in_=x_in[0, :, i * STEP : (i + 1) * STEP])
                xt_pre.append(xt)

            w_sb = consts.tile([D, 4, 2 * K], F16)
            nc.sync.dma_start(out=w_sb, in_=w_in[:, :, :])
            ws = [w_sb[:, i, :] for i in range(4)]
            cc_sb = consts.tile([2 * K, 1], F32)
            nc.sync.dma_start(out=cc_sb, in_=cc_in[:, :])
            ones_bd = consts.tile([2 * K, 2 * K], F16)
            nc.sync.dma_start(out=ones_bd, in_=ones_in[:, :])
            half_sb = consts.tile([D, 1], F32)
            nc.gpsimd.memset(half_sb, 0.5)

            n_iters = N // STEP  # 8 per batch row
            iters = [(b, q) for b in range(BPC) for q in range(n_iters)]
            NI = len(iters)
            st = [dict() for _ in range(NI)]

            # software-pipelined emission: each engine's in-order stream
            # interleaves stages of consecutive iterations so no stage
            # head-of-line-blocks the next iteration's earlier stage
            def s0_load(i):
                if i < 2:
                    st[i]["xt"] = xt_pre[i]
                    return
                b, q = iters[i]
                n0 = q * STEP
                xt = xtp.tile([D, STEP], F32R, tag="xt")
                nc.sync.dma_start(out=xt, in_=x_in[b, :, n0 : n0 + STEP])
                st[i]["xt"] = xt

            # q-pass engine schedule, balanced against measured per-op cost
            # (Act Square 1.9us, DVE mul 2.9us, Pool mul 4.0us) and each
            # engine's other duties.  Pool (no epilogue duties) takes the
            # early/even iterations and is done by mid-run; the LAST three
            # iterations compute q as two half-tiles in PARALLEL on DVE+Act
            # so the tail's q latency is ~1.5us instead of a 4us Pool op at
            # the end of Pool's serial backlog.
            q_eng = {0: "p", 1: "d", 2: "p", 3: "d", 4: "p", 5: "a",
                     6: "p", 7: "d", 8: "p", 9: "d", 10: "p", 11: "a",
                     12: "p", 13: "p", 14: "d", 15: "d"}

            def s1_q(i):
                xt = st[i]["xt"]
                qt = xp.tile([D, STEP], F16, tag="qt")
                # q = x*x; tensor_tensor multiply is one of the few ops the
                # GPSIMD ucode implements, so Pool can take a large share
                e = q_eng[i % 16]
                if e == "s":
                    # tail iterations: two halves in parallel on DVE + Pool
                    # (Pool's serial q backlog is drained by then)
                    h = slice(0, PAIR)
                    nc.vector.tensor_mul(
                        qt[:, h], xt.bitcast(F32)[:, h], xt.bitcast(F32)[:, h]
                    )
                    h = slice(PAIR, STEP)
                    nc.gpsimd.tensor_mul(
                        qt[:, h], xt.bitcast(F32)[:, h], xt.bitcast(F32)[:, h]
                    )
                elif e == "a":
                    nc.scalar.activation(
                        out=qt, in_=xt.bitcast(F32),
                        func=mybir.ActivationFunctionType.Square,
                    )
                else:
                    eng = nc.gpsimd if e == "p" else nc.vector
                    eng.tensor_mul(qt, xt.bitcast(F32), xt.bitcast(F32))
                st[i]["qt"] = qt

            def s1_u(i):
                xt = st[i]["xt"]
                ut = xp.tile([D, STEP], F16, tag="ut")
                # u = (x + 0.5)^2 = x^2 + x + 0.25 in ONE ScalarE op via the
                # activation's free affine pre-add (bias)
                nc.scalar.activation(
                    out=ut, in_=xt.bitcast(F32),
                    func=mybir.ActivationFunctionType.Square,
                    bias=half_sb, scale=1.0,
                )
                st[i]["ut"] = ut

            def s2_dist(i):
                qt, ut = st[i]["qt"], st[i]["ut"]
                # stacked dist GEMM: tile A (cols 0:1024) lands on PSUM
                # partitions 0-63 via [w;0] weights, tile B (cols 1024:2048)
                # on partitions 64-127 via [0;w], accumulated.  Each 512-col
                # bank gets its OWN single-bank psum tile so the epilogue
                # chains per bank (psum pools run at depth 4 instead of 2
                # and exp(h0) starts while bank h1 still accumulates).
                pds = []
                for h in range(2):
                    pd_t = pdp.tile([2 * K, NT], F32, tag="pd")
                    movs = [qt, ut, qt, ut]
                    offs = [0, 0, PAIR, PAIR]
                    for wi in range(4):
                        msl = slice(
                            offs[wi] + h * NT, offs[wi] + (h + 1) * NT
                        )
                        nc.tensor.matmul(
                            pd_t[:, :], ws[wi], movs[wi][:, msl],
                            start=(wi == 0), stop=(wi == 3),
                        )
                    pds.append(pd_t)
                st[i]["pd"] = pds

            def s3_exp(i):
                pds = st[i].pop("pd")
                ets = []
                for h in range(2):
                    et = ep.tile([2 * K, NT], F16, tag=f"et{h}")
                    nc.scalar.activation(
                        out=et, in_=pds[h],
                        func=mybir.ActivationFunctionType.Exp,
                        bias=cc_sb, scale=1.0,
                    )
                    ets.append(et)
                st[i]["et"] = ets
                st[i].pop("xt")
                st[i].pop("qt")
                st[i].pop("ut")

            def s4_den(i):
                ets = st[i]["et"]
                # denominator, summed over each 64-partition block AND
                # broadcast within the block in one shot: blockdiag ones
                pbs = []
                for h in range(2):
                    pb_t = pbp.tile([2 * K, NT], F32, tag="pb")
                    nc.tensor.matmul(
                        pb_t[:, :], ones_bd, ets[h][:, :],
                        start=True, stop=True,
                    )
                    pbs.append(pb_t)
                st[i]["pb"] = pbs

            def s5_recip(i):
                pbs = st[i].pop("pb")
                rs = []
                for h in range(2):
                    r16 = rp.tile([2 * K, NT], F16, tag=f"r{h}")
                    # ~18-bit-accurate custom-DVE reciprocal with direct fp16
                    # write-back; the sum is always >= e^-7 (mean-centered,
                    # -7-shifted logits), so the undefined edge cases
                    # (0/denorm/inf) cannot occur
                    c = RECIP_APPROX_FAST_CONSTS
                    nc.vector._custom_dve(
                        RECIPROCAL_APPROX_FAST, out=r16, in0=pbs[h],
                        s0=c["s0"], s1=c["s1"], imm2=c["imm2"],
                    )
                    rs.append(r16)
                st[i]["r"] = rs

            def s6_mult(i):
                ets, rs = st[i].pop("et"), st[i].pop("r")
                ot = op.tile([2 * K, PAIR], F16, tag="ot")
                # all-fp16 SBUF operands -> DVE 2x_1p mode
                for h in range(2):
                    sl = slice(h * NT, (h + 1) * NT)
                    nc.vector.tensor_mul(ot[:, sl], ets[h], rs[h])
                st[i]["ot"] = ot

            def s7_store(i):
                b, q = iters[i]
                n0 = q * STEP
                ot = st[i].pop("ot")
                # early stores go out on the Activation HWDGE queue: on the
                # SP queue their wait on mul(i) would gate every later x
                # load (the SP stream is in-order), capping load issue at
                # the compute-chain rate.  LATE stores (i >= 10) return to
                # SP, which has finished issuing loads by then - otherwise
                # the tail's store issues pile up behind the final exps in
                # the Act queue and drain serially after the last compute.
                eng = nc.sync if i >= 10 else nc.scalar
                eng.dma_start(
                    out=out_ext[b, :, n0 : n0 + PAIR], in_=ot[0:K, :]
                )
                eng.dma_start(
                    out=out_ext[b, :, n0 + PAIR : n0 + STEP],
                    in_=ot[K : 2 * K, :],
                )

            stages = [
                s0_load, s1_u, s1_q, s2_dist, s3_exp,
                s4_den, s5_recip, s6_mult, s7_store,
            ]
            NS = len(stages)
            # Per-tick emission order tuned per engine queue:
            #  - load (k=0) first: on the in-order SP queue a store's
            #    semaphore wait on mul(i) would otherwise head-of-line-block
            #    every later x load, collapsing HBM utilization.
            #  - u (k=2) BEFORE exp (k=4): both run on ScalarE, and u's
            #    input is a load (ready early) while exp waits on the dist
            #    matmul.  With exp first, the tail serializes into an
            #    exp->u->matmul->exp ping-pong at ~4.5us/iteration; with u
            #    first ScalarE pre-computes the u's and the tail exps flow.
            #  - otherwise downstream stages first so no engine's in-order
            #    queue blocks a later iteration's earlier stage.
            # stores (k=8) right after the mul stage: on the Act queue they
            # then precede the tick's u/exp, issuing as soon as mul(i) lands
            # instead of queueing behind four more iterations of exps.
            # u (k=1, skew 1) and q (k=2) run as close to their load as
            # possible so the tail's front passes never wait on epilogues.
            for tick in range(NI + NS - 1):
                for k in [0, 7, 8, 6, 1, 2, 5, 4, 3]:
                    i = tick - k
                    if 0 <= i < NI:
                        stages[k](i)
    nc.compile()
    return nc


def _host_params(mu, log_sigma, log_alpha):
    mu64 = mu.astype(np.float64)
    mu_n = mu64 / np.maximum(
        np.linalg.norm(mu64, axis=1, keepdims=True), 1e-12
    )
    sinv = np.exp(-log_sigma.astype(np.float64))  # (K, D)
    a1 = -sinv                                    # coeff of x^2 in logits
    a2 = 2.0 * mu_n * sinv                        # coeff of x
    c = (
        -np.sum(mu_n * mu_n * sinv, axis=1)
        + log_alpha.astype(np.float64)
        - 0.5 * np.sum(log_sigma.astype(np.float64), axis=1)
    )
    # change of basis: logits = sum_d wq*q + wu*u + c' with q = x^2 and
    # u = (x+0.5)^2 = x^2 + x + 0.25 (so x = u - q - 0.25)
    wq = a1 - a2
    wu = a2
    cb = c - 0.25 * np.sum(a2, axis=1)
    # center across K (softmax is invariant to per-n shifts; keeps logits
    # within ~+-16), then shift by -7 so e = exp(logit) fits in fp16
    wqc = wq - wq.mean(axis=0, keepdims=True)
    wuc = wu - wu.mean(axis=0, keepdims=True)
    ccv = cb - cb.mean() - 7.0
    w1 = np.ascontiguousarray(wqc.T, dtype=np.float32)  # (D, K)
    w2 = np.ascontiguousarray(wuc.T, dtype=np.float32)  # (D, K)
    # zero-padded stationaries for the partition-stacked dist GEMM
    z = np.zeros_like(w1)
    w = np.stack(
        [
            np.concatenate([w1, z], axis=1),  # wq_lo: PE cols 0-63
            np.concatenate([w2, z], axis=1),  # wu_lo
            np.concatenate([z, w1], axis=1),  # wq_hi: PE cols 64-127
            np.concatenate([z, w2], axis=1),  # wu_hi
        ],
        axis=1,
    )  # (D, 4, 2K)
    cc = ccv.astype(np.float32).reshape(K, 1)
    cc2 = np.concatenate([cc, cc], axis=0)  # (2K, 1)
    return np.ascontiguousarray(w).astype(np.float16), cc2


def _in_maps(x, mu, log_sigma, log_alpha):
    x = np.ascontiguousarray(np.asarray(x), dtype=np.float32)
    w, cc2 = _host_params(
        np.asarray(mu), np.asarray(log_sigma), np.asarray(log_alpha)
    )
    eye2 = np.kron(
        np.eye(2, dtype=np.float32), np.ones((K, K), np.float32)
    ).astype(np.float16)
    return [
        {
            "x": x[i * BPC : (i + 1) * BPC],
            "w": w,
            "cc": cc2,
            "ones_bd": eye2,
        }
        for i in range(NCORES)
    ]


def kernel(x, mu, log_sigma, log_alpha):
    if "nc" not in _CACHE:
        _CACHE["nc"] = _build_nc()
    nc = _CACHE["nc"]
    in_maps = _in_maps(x, mu, log_sigma, log_alpha)
    res = run_bass_kernel_spmd(nc, in_maps, list(range(NCORES))).results
    out = np.concatenate(
        [np.asarray(res[i]["out"]) for i in range(NCORES)], axis=0
    )
    return out.astype(np.float32)


# revision 52
# speedup vs baseline: 1.0154x; 1.0154x over previous
"""Trainium2 Bass kernel for DiagonalGMMPosterior (vq_codebook).

Reference computation (per batch b, descriptor n, cluster k):
    dist[k,n]  = sum_d (x[d,n] - mu_n[k,d])^2 * exp(-log_sigma[k,d])
    logits     = -dist + log_alpha[k] - 0.5 * sum_d log_sigma[k,d]
    out[k,n]   = softmax_k(logits)

Device strategy (8 NeuronCores, data-parallel over the batch axis):
  * Host folds all (K,D) parameter math into GEMM coefficients, CENTERS
    them across K (softmax is shift invariant per column -> logits stay
    within ~+-16, no per-n max pass), then shifts by -7 so exp() fits
    comfortably in fp16 (max e ~ e^9; values that underflow fp16 have
    posterior < 1e-4, far under tolerance).
  * The GEMM uses two quadratic bases computed from x in one elementwise
    pass each, avoiding any separate f32->f16 conversion of x:
        q = x^2           (ScalarE Square, fp16 out)
        u = x^2 + 0.5 x   (DVE/GpSimd scalar_tensor_tensor, fp16 out)
    logits = sum_d (a1-2*a2)*q + (2*a2)*u + cc.
  * All matmuls are fp16 (16-bit stationaries share LDWEIGHTS across
    matmuls; f32r must reload per-matmul which serializes TensorE).
  * K=64 uses half the 128 lanes and ScalarE/VectorE cost is per-COLUMN,
    so each iteration processes TWO 1024-column tiles stacked on the
    partition axis: tile A's dist GEMM uses zero-padded stationaries
    [w;0] (PE cols 0-63), tile B [0;w] (PE cols 64-127), accumulating
    into the SAME PSUM banks.  The whole epilogue (exp, block-diag ones
    denominator matmul, reciprocal, normalize) runs on [128, 1024] tiles.
  * The reciprocal writes fp16 directly (RECIPROCAL_APPROX_FAST via
    _custom_dve; the fp32-only wrapper assert guards the in-pipe bit
    trick, the output cast is the standard DVE write port) so the final
    multiply is all-16-bit and hits the DVE 2x mode.
"""

import ml_dtypes
import numpy as np

import concourse.bacc as bacc
import concourse.bass as bass
import concourse.tile as tile
from concourse import mybir
from concourse.bass_utils import run_bass_kernel_spmd
from concourse.dve_ops import RECIP_APPROX_FAST_CONSTS, RECIPROCAL_APPROX_FAST

B, D, N, K = 16, 128, 16384, 64
NCORES = 8
BPC = B // NCORES   # batches per core
NT = 512            # one PSUM bank of fp32
PAIR = 2 * NT       # psum tile width (columns shared by the stacked halves)
STEP = 2 * PAIR     # x columns consumed per iteration (two 1024-col tiles)

F32 = mybir.dt.float32
F32R = mybir.dt.float32r
F16 = mybir.dt.float16

_CACHE = {}


def _build_nc():
    # Bacc (not raw Bass): its compile() pass legalizes Tile's multi-wait
    # instructions (move_matmul_waits_to_ldweights + generate_event_semaphores)
    # down to the 1-wait-per-instruction hardware limit.
    nc = bacc.Bacc("TRN2", target_bir_lowering=False, debug=False)
    x_in = nc.declare_dram_parameter("x", [BPC, D, N], F32R, isOutput=False)
    # four zero-padded fp16 stationaries: [wq;0], [wu;0], [0;wq], [0;wu]
    w_in = nc.declare_dram_parameter("w", [D, 4, 2 * K], F16, isOutput=False)
    cc_in = nc.declare_dram_parameter("cc", [2 * K, 1], F32, isOutput=False)
    ones_in = nc.declare_dram_parameter(
        "ones_bd", [2 * K, 2 * K], F16, isOutput=False
    )
    # fp16 output halves the store traffic at ~5e-4 rounding (posteriors
    # live in [0,1]); the host widens back to fp32
    out_ext = nc.declare_dram_parameter("out", [BPC, K, N], F16, isOutput=True)

    with tile.TileContext(nc) as tc:
        with (
            tc.tile_pool(name="consts", bufs=1) as consts,
            tc.tile_pool(name="xtp", bufs=12) as xtp,
            tc.tile_pool(name="xp", bufs=6) as xp,
            tc.tile_pool(name="ep", bufs=6) as ep,
            tc.tile_pool(name="op", bufs=6) as op,
            tc.tile_pool(name="rp", bufs=6) as rp,
            tc.tile_pool(name="pd", bufs=4, space="PSUM") as pdp,
            tc.tile_pool(name="pb", bufs=4, space="PSUM") as pbp,
        ):
            # x loads for the first two iterations go on the SP queue BEFORE
            # the consts so the first compute starts ~2us earlier; the
            # consts are only needed by the matmul/exp stages
            xt_pre = []
            for i in range(2):
                xt = xtp.tile([D, STEP], F32R, tag="xt")
                nc.sync.dma_start(out=xt, in_=x_in[0, :, i * STEP : (i + 1) * STEP])
                xt_pre.append(xt)

            w_sb = consts.tile([D, 4, 2 * K], F16)
            nc.sync.dma_start(out=w_sb, in_=w_in[:, :, :])
            ws = [w_sb[:, i, :] for i in range(4)]
            cc_sb = consts.tile([2 * K, 1], F32)
            nc.sync.dma_start(out=cc_sb, in_=cc_in[:, :])
            ones_bd = consts.tile([2 * K, 2 * K], F16)
            nc.sync.dma_start(out=ones_bd, in_=ones_in[:, :])
            half_sb = consts.tile([D, 1], F32)
            nc.gpsimd.memset(half_sb, 0.5)

            n_iters = N // STEP  # 8 per batch row
            iters = [(b, q) for b in range(BPC) for q in range(n_iters)]
            NI = len(iters)
            st = [dict() for _ in range(NI)]

            # software-pipelined emission: each engine's in-order stream
            # interleaves stages of consecutive iterations so no stage
            # head-of-line-blocks the next iteration's earlier stage
            def s0_load(i):
                if i < 2:
                    st[i]["xt"] = xt_pre[i]
                    return
                b, q = iters[i]
                n0 = q * STEP
                xt = xtp.tile([D, STEP], F32R, tag="xt")
                nc.sync.dma_start(out=xt, in_=x_in[b, :, n0 : n0 + STEP])
                st[i]["xt"] = xt

            # q-pass engine schedule, balanced against measured per-op cost
            # (Act Square 1.9us, DVE mul 2.9us, Pool mul 4.0us) and each
            # engine's other duties.  Pool (no epilogue duties) takes the
            # early/even iterations and is done by mid-run; the LAST three
            # iterations compute q as two half-tiles in PARALLEL on DVE+Act
            # so the tail's q latency is ~1.5us instead of a 4us Pool op at
            # the end of Pool's serial backlog.
            q_eng = {0: "p", 1: "d", 2: "p", 3: "d", 4: "p", 5: "a",
                     6: "p", 7: "d", 8: "p", 9: "d", 10: "p", 11: "a",
                     12: "p", 13: "p", 14: "d", 15: "d"}

            def s1_q(i):
                xt = st[i]["xt"]
                qt = xp.tile([D, STEP], F16, tag="qt")
                # q = x*x; tensor_tensor multiply is one of the few ops the
                # GPSIMD ucode implements, so Pool can take a large share
                e = q_eng[i % 16]
                if e == "s":
                    # tail iterations: two halves in parallel on DVE + Pool
                    # (Pool's serial q backlog is drained by then)
                    h = slice(0, PAIR)
                    nc.vector.tensor_mul(
                        qt[:, h], xt.bitcast(F32)[:, h], xt.bitcast(F32)[:, h]
                    )
                    h = slice(PAIR, STEP)
                    nc.gpsimd.tensor_mul(
                        qt[:, h], xt.bitcast(F32)[:, h], xt.bitcast(F32)[:, h]
                    )
                elif e == "a":
                    nc.scalar.activation(
                        out=qt, in_=xt.bitcast(F32),
                        func=mybir.ActivationFunctionType.Square,
                    )
                else:
                    eng = nc.gpsimd if e == "p" else nc.vector
                    eng.tensor_mul(qt, xt.bitcast(F32), xt.bitcast(F32))
                st[i]["qt"] = qt

            def s1_u(i):
                xt = st[i]["xt"]
                ut = xp.tile([D, STEP], F16, tag="ut")
                # u = (x + 0.5)^2 = x^2 + x + 0.25 in ONE ScalarE op via the
                # activation's free affine pre-add (bias)
                nc.scalar.activation(
                    out=ut, in_=xt.bitcast(F32),
                    func=mybir.ActivationFunctionType.Square,
                    bias=half_sb, scale=1.0,
                )
                st[i]["ut"] = ut

            def s2_dist(i):
                qt, ut = st[i]["qt"], st[i]["ut"]
                # stacked dist GEMM: tile A (cols 0:1024) lands on PSUM
                # partitions 0-63 via [w;0] weights, tile B (cols 1024:2048)
                # on partitions 64-127 via [0;w], accumulated.  Each 512-col
                # bank gets its OWN single-bank psum tile so the epilogue
                # chains per bank (psum pools run at depth 4 instead of 2
                # and exp(h0) starts while bank h1 still accumulates).
                pds = []
                for h in range(2):
                    pd_t = pdp.tile([2 * K, NT], F32, tag="pd")
                    movs = [qt, ut, qt, ut]
                    offs = [0, 0, PAIR, PAIR]
                    for wi in range(4):
                        msl = slice(
                            offs[wi] + h * NT, offs[wi] + (h + 1) * NT
                        )
                        nc.tensor.matmul(
                            pd_t[:, :], ws[wi], movs[wi][:, msl],
                            start=(wi == 0), stop=(wi == 3),
                        )
                    pds.append(pd_t)
                st[i]["pd"] = pds

            def s3_exp(i):
                pds = st[i].pop("pd")
                ets = []
                for h in range(2):
                    et = ep.tile([2 * K, NT], F16, tag=f"et{h}")
                    nc.scalar.activation(
                        out=et, in_=pds[h],
                        func=mybir.ActivationFunctionType.Exp,
                        bias=cc_sb, scale=1.0,
                    )
                    ets.append(et)
                st[i]["et"] = ets
                st[i].pop("xt")
                st[i].pop("qt")
                st[i].pop("ut")

            def s4_den(i):
                ets = st[i]["et"]
                # denominator, summed over each 64-partition block AND
                # broadcast within the block in one shot: blockdiag ones
                pbs = []
                for h in range(2):
                    pb_t = pbp.tile([2 * K, NT], F32, tag="pb")
                    nc.tensor.matmul(
                        pb_t[:, :], ones_bd, ets[h][:, :],
                        start=True, stop=True,
                    )
                    pbs.append(pb_t)
                st[i]["pb"] = pbs

            def s5_recip(i):
                pbs = st[i].pop("pb")
                rs = []
                for h in range(2):
                    r16 = rp.tile([2 * K, NT], F16, tag=f"r{h}")
                    # ~18-bit-accurate custom-DVE reciprocal with direct fp16
                    # write-back; the sum is always >= e^-7 (mean-centered,
                    # -7-shifted logits), so the undefined edge cases
                    # (0/denorm/inf) cannot occur
                    c = RECIP_APPROX_FAST_CONSTS
                    nc.vector._custom_dve(
                        RECIPROCAL_APPROX_FAST, out=r16, in0=pbs[h],
                        s0=c["s0"], s1=c["s1"], imm2=c["imm2"],
                    )
                    rs.append(r16)
                st[i]["r"] = rs

            def s6_mult(i):
                ets, rs = st[i].pop("et"), st[i].pop("r")
                ot = op.tile([2 * K, PAIR], F16, tag="ot")
                # all-fp16 SBUF operands -> DVE 2x_1p mode
                for h in range(2):
                    sl = slice(h * NT, (h + 1) * NT)
                    nc.vector.tensor_mul(ot[:, sl], ets[h], rs[h])
                st[i]["ot"] = ot

            def s7_store(i):
                b, q = iters[i]
                n0 = q * STEP
                ot = st[i].pop("ot")
                # early stores go out on the Activation HWDGE queue: on the
                # SP queue their wait on mul(i) would gate every later x
                # load (the SP stream is in-order), capping load issue at
                # the compute-chain rate.  LATE stores (i >= 10) return to
                # SP, which has finished issuing loads by then - otherwise
                # the tail's store issues pile up behind the final exps in
                # the Act queue and drain serially after the last compute.
                eng = nc.sync if i >= 10 else nc.scalar
                eng.dma_start(
                    out=out_ext[b, :, n0 : n0 + PAIR], in_=ot[0:K, :]
                )
                eng.dma_start(
                    out=out_ext[b, :, n0 + PAIR : n0 + STEP],
                    in_=ot[K : 2 * K, :],
                )

            stages = [
                s0_load, s1_q, s1_u, s2_dist, s3_exp,
                s4_den, s5_recip, s6_mult, s7_store,
            ]
            NS = len(stages)
            # Per-tick emission order tuned per engine queue:
            #  - load (k=0) first: on the in-order SP queue a store's
            #    semaphore wait on mul(i) would otherwise head-of-line-block
            #    every later x load, collapsing HBM utilization.
            #  - u (k=2) BEFORE exp (k=4): both run on ScalarE, and u's
            #    input is a load (ready early) while exp waits on the dist
            #    matmul.  With exp first, the tail serializes into an
            #    exp->u->matmul->exp ping-pong at ~4.5us/iteration; with u
            #    first ScalarE pre-computes the u's and the tail exps flow.
            #  - otherwise downstream stages first so no engine's in-order
            #    queue blocks a later iteration's earlier stage.
            # stores (k=8) right after the mul stage: on the Act queue they
            # then precede the tick's u/exp, issuing as soon as mul(i) lands
            # instead of queueing behind four more iterations of exps
            for tick in range(NI + NS - 1):
                for k in [0, 7, 8, 6, 2, 5, 4, 3, 1]:
                    i = tick - k
                    if 0 <= i < NI:
                        stages[k](i)
    nc.compile()
    return nc


def _host_params(mu, log_sigma, log_alpha):
    mu64 = mu.astype(np.float64)
    mu_n = mu64 / np.maximum(
        np.linalg.norm(mu64, axis=1, keepdims=True), 1e-12
    )
    sinv = np.exp(-log_sigma.astype(np.float64))  # (K, D)
    a1 = -sinv                                    # coeff of x^2 in logits
    a2 = 2.0 * mu_n * sinv                        # coeff of x
    c = (
        -np.sum(mu_n * mu_n * sinv, axis=1)
        + log_alpha.astype(np.float64)
        - 0.5 * np.sum(log_sigma.astype(np.float64), axis=1)
    )
    # change of basis: logits = sum_d wq*q + wu*u + c' with q = x^2 and
    # u = (x+0.5)^2 = x^2 + x + 0.25 (so x = u - q - 0.25)
    wq = a1 - a2
    wu = a2
    cb = c - 0.25 * np.sum(a2, axis=1)
    # center across K (softmax is invariant to per-n shifts; keeps logits
    # within ~+-16), then shift by -7 so e = exp(logit) fits in fp16
    wqc = wq - wq.mean(axis=0, keepdims=True)
    wuc = wu - wu.mean(axis=0, keepdims=True)
    ccv = cb - cb.mean() - 7.0
    w1 = np.ascontiguousarray(wqc.T, dtype=np.float32)  # (D, K)
    w2 = np.ascontiguousarray(wuc.T, dtype=np.float32)  # (D, K)
    # zero-padded stationaries for the partition-stacked dist GEMM
    z = np.zeros_like(w1)
    w = np.stack(
        [
            np.concatenate([w1, z], axis=1),  # wq_lo: PE cols 0-63
            np.concatenate([w2, z], axis=1),  # wu_lo
            np.concatenate([z, w1], axis=1),  # wq_hi: PE cols 64-127
            np.concatenate([z, w2], axis=1),  # wu_hi
        ],
        axis=1,
    )  # (D, 4, 2K)
    cc = ccv.astype(np.float32).reshape(K, 1)
    cc2 = np.concatenate([cc, cc], axis=0)  # (2K, 1)
    return np.ascontiguousarray(w).astype(np.float16), cc2


def _in_maps(x, mu, log_sigma, log_alpha):
    x = np.ascontiguousarray(np.asarray(x), dtype=np.float32)
    w, cc2 = _host_params(
        np.asarray(mu), np.asarray(log_sigma), np.asarray(log_alpha)
    )
    eye2 = np.kron(
        np.eye(2, dtype=np.float32), np.ones((K, K), np.float32)
    ).astype(np.float16)
    return [
        {
            "x": x[i * BPC : (i + 1) * BPC],
            "w": w,
            "cc": cc2,
            "ones_bd": eye2,
        }
        for i in range(NCORES)
    ]


def kernel(x, mu, log_sigma, log_alpha):
    if "nc" not in _CACHE:
        _CACHE["nc"] = _build_nc()
    nc = _CACHE["nc"]
    in_maps = _in_maps(x, mu, log_sigma, log_alpha)
    res = run_bass_kernel_spmd(nc, in_maps, list(range(NCORES))).results
    out = np.concatenate(
        [np.asarray(res[i]["out"]) for i in range(NCORES)], axis=0
    )
    return out.astype(np.float32)
